# revision 39
# baseline (speedup 1.0000x reference)
"""BERT self-attention (B=4, S=2048, H=1024, 16 heads x 64) on 8 TRN2 NeuronCores.

Sharding: data-parallel over batch (4) x tensor-parallel over head-groups (2).
Core c handles batch c//2 and heads [8*(c%2), 8*(c%2)+8): it gets the full
hidden_states[b] plus the 512 W-columns/bias entries for its heads, and
produces out[b, :, 512*g : 512*(g+1)]. No cross-core communication.

Per-core kernel (bf16/fp16 matmuls, f32 accumulation in PSUM):
  phase1: all weight DMAs issued upfront (wv first) on the gpsimd queue,
    all x tiles on the sync queue. x is transposed by the PE in f32
    (2 cyc/row); the DVE PSUM->SBUF drain does the f32->bf16 cast for
    free, so the scalar engine (ACT) does no phase-1 work at all.
    Weight casts f32->bf16 run on the DVE, ordered wv -> st-chain -> wq/wk
    so they never delay the xT pipeline.
  attention (per head-pair, q-macro 512, k-chunk 128):
    scoresT[k, q] = KT_h[:, kc].T @ QT_h[:, qm]   (two heads row-packed, K=64)
    expT = exp(0.125 * scoresT)                   (ACT, fp16 out, N=1024/inst)
    ctxT[65, q] += V'_h[kc].T @ expT              (row 64 = softmax denominator)
    + one Q/K projection matmul of the NEXT head-pair in the PE slack.
    The loop is ACT-paced at ~1.34us/k-chunk (256 activations is the floor
    given the 8-bank PSUM budget: sc 2x2 + ctx 2x1 + pj 2x1).
  epilogue: ctxT -> DMA-transpose -> [q, 65]; reciprocal-multiply by the
  denominator; one batched [128, 128] DMA per q-slice covering both heads,
  alternating between the sync and gpsimd queues. (Final q-macro uses a PE
  transpose to keep the kernel tail short.)
"""

import sys
import types

sys.path.insert(0, "/opt/trn_rl_repo")

import numpy as np

import concourse.bass as bass
import concourse.tile as tile
from concourse import bacc, mybir
from concourse.bass_utils import run_bass_kernel_spmd
from concourse.masks import make_identity

B, S, H = 4, 2048, 1024
NH, HD = 16, 64
NCORES = 8
HEADS_PER_CORE = NH // 2      # 8 heads per core
HG = HEADS_PER_CORE * HD      # 512 = per-core head width
P = 128
QM = 512                      # q macro-tile
N_QM = S // QM                # 4
N_KC = S // P                 # 16 k chunks
N_ST = S // P                 # 16 s tiles
N_HB = H // P                 # 8 h chunks (contraction)
N_MT = HG // P                # 4 hd m-tiles

FP32 = mybir.dt.float32
BF16 = mybir.dt.bfloat16
FP16 = mybir.dt.float16


def _ensure_profile_hook():
    """The image's antenv lacks axon_hooks; shim it so trace=True works."""
    try:
        from antenv.axon_hooks import get_axon_ntff_profile_hook  # noqa: F401
        return
    except ImportError:
        pass
    try:
        from trn_agent_boot.trn_boot import _ntff_profile_via_ctypes
    except ImportError:
        return
    hook = _ntff_profile_via_ctypes("/opt/axon/libaxon_pjrt.so")
    mod = types.ModuleType("antenv.axon_hooks")
    mod.get_axon_ntff_profile_hook = lambda: hook
    mod.set_axon_ntff_profile_hook = lambda h: None
    sys.modules["antenv.axon_hooks"] = mod


def build():
    nc = bacc.Bacc("TRN2", target_bir_lowering=False, debug=False,
                   num_devices=NCORES)

    x_d = nc.declare_dram_parameter("x", [S, H], FP32, isOutput=False)
    wq_d = nc.declare_dram_parameter("wq", [H, HG], FP32, isOutput=False)
    wk_d = nc.declare_dram_parameter("wk", [H, HG], FP32, isOutput=False)
    wv_d = nc.declare_dram_parameter("wv", [H, HG], FP32, isOutput=False)
    bq_d = nc.declare_dram_parameter("bq", [HG], FP32, isOutput=False)
    bk_d = nc.declare_dram_parameter("bk", [HG], FP32, isOutput=False)
    bv_d = nc.declare_dram_parameter("bv", [HG], FP32, isOutput=False)
    out_d = nc.declare_dram_parameter("out", [S, HG], FP32, isOutput=True)

    with tile.TileContext(nc) as tc:
        _build_body(nc, tc, x_d, (wq_d, wk_d, wv_d), (bq_d, bk_d, bv_d), out_d)

    nc.finalize()
    return nc


def _build_body(nc, tc, x_d, w_d, b_d, out_d):
    wq_d, wk_d, wv_d = w_d
    bq_d, bk_d, bv_d = b_d

    import contextlib
    ctx = contextlib.ExitStack()
    with ctx:
        const = ctx.enter_context(tc.tile_pool(name="const", bufs=1))
        xf = ctx.enter_context(tc.tile_pool(name="xf", bufs=5))
        big = ctx.enter_context(tc.tile_pool(name="big", bufs=1))
        wstage = ctx.enter_context(tc.tile_pool(name="wstage", bufs=10))
        expp = ctx.enter_context(tc.tile_pool(name="expp", bufs=12))
        epil = ctx.enter_context(tc.tile_pool(name="epil", bufs=3))
        outp = ctx.enter_context(tc.tile_pool(name="outp", bufs=8))
        # created last: the exp tiles' SBUF address (expp) is performance-
        # critical (ACT activation speed is layout sensitive), so new pools
        # must not shift it.
        xbp = ctx.enter_context(tc.tile_pool(name="xbp", bufs=4))
        # PSUM budget (8 banks): ps_sc = 2 x 2-bank slots (scores double
        # buffer; also hosts the f32 transpose tiles in phase 1), ps_ctx =
        # 2 x 1-bank slots (ctx accumulators), ps_pj = 2 x 1-bank slots
        # (V'/QK projection accumulators).
        ps_sc = ctx.enter_context(
            tc.tile_pool(name="ps_sc", bufs=2, space="PSUM"))
        ps_ctx = ctx.enter_context(
            tc.tile_pool(name="ps_ctx", bufs=2, space="PSUM"))
        ps_pj = ctx.enter_context(
            tc.tile_pool(name="ps_pj", bufs=2, space="PSUM"))

        # ---- constants -------------------------------------------------
        ident_f = const.tile([P, P], FP32)
        make_identity(nc, ident_f)
        ident_h = const.tile([HD + 1, HD + 1], FP16)
        make_identity(nc, ident_h)

        # ---- x: tiles 0-4 lead on the sync queue; 5-15 are queued on the
        # gpsimd queue BEHIND the weights, so the early HBM bandwidth
        # concentrates on wq/wk (which gate the first attention q-macro).
        xt_tiles = []
        for st in range(5):
            xt = xf.tile([P, H], FP32, tag="x", name=f"x{st}")
            nc.sync.dma_start(out=xt, in_=x_d.ap()[st * P:(st + 1) * P, :])
            xt_tiles.append(xt)

        bqT = const.tile([P, N_MT], FP32)
        nc.gpsimd.dma_start(out=bqT, in_=bq_d.ap().rearrange("(o p) -> p o", p=P))
        bkT = const.tile([P, N_MT], FP32)
        nc.gpsimd.dma_start(out=bkT, in_=bk_d.ap().rearrange("(o p) -> p o", p=P))
        bv_ap = bv_d.ap()
        bvb = const.tile([P, HG], FP32)
        nc.gpsimd.dma_start(
            out=bvb,
            in_=bass.AP(tensor=bv_ap.tensor, offset=bv_ap.offset,
                        ap=[[0, P]] + [list(a) for a in bv_ap.ap]),
        )

        # ---- weights: every DMA is issued upfront on the gpsimd queue.
        # wv comes first in row-chunks (V' consumes it within microseconds).
        # wq/wk are transferred in head-pair column slices so that only the
        # hp0 slice gates the first attention q-macro; the other slices
        # stream and cast (on the DVE) underneath the merged q-macro.
        w_sb = {}
        w_stg = {}
        for name in ("v", "q", "k"):
            w_sb[name] = big.tile([P, N_HB, HG], BF16, tag=f"w{name}",
                                  name=f"w{name}")

        # row-chunk DMAs (2KB lines -- column slices would be ~10x slower
        # per byte). wq/wk first: they gate the first attention q-macro.
        for name, wd in (("q", wq_d), ("k", wk_d), ("v", wv_d)):
            for hb in range(N_HB):
                stg = wstage.tile([P, HG], FP32, tag="wstg", bufs=10,
                                  name=f"w{name}{hb}")
                nc.gpsimd.dma_start(
                    out=stg, in_=wd.ap()[hb * P:(hb + 1) * P, :])
                w_stg[(name, hb)] = stg

        for st in range(5, N_ST):
            xt = xf.tile([P, H], FP32, tag="x", name=f"x{st}")
            nc.gpsimd.dma_start(out=xt, in_=x_d.ap()[st * P:(st + 1) * P, :])
            xt_tiles.append(xt)

        def cast_w(name):
            for hb in range(N_HB):
                nc.vector.tensor_copy(out=w_sb[name][:, hb, :],
                                      in_=w_stg.pop((name, hb)))

        xT = big.tile([P, N_HB, S], BF16, tag="xT")
        vp = big.tile([P, N_ST, HEADS_PER_CORE, HD + 1], BF16, tag="vp")
        # only the denominator ones-column needs initializing; columns 0:HD
        # are fully overwritten by the V' bias-add drains.
        nc.vector.memset(vp[:, :, :, HD:HD + 1], 1.0)

        qT = big.tile([P, N_MT, S], BF16, tag="qT")
        kT = big.tile([P, N_MT, S], BF16, tag="kT")

        def proj_chunk(mt, n, pool=None, tag=None, nbufs=None):
            pool = pool or ps_pj
            tag = tag or "pjp"
            if pool is ps_pj:
                nbufs = 1
            for w_name, dst, bias in (("q", qT, bqT), ("k", kT, bkT)):
                ps = pool.tile([P, QM], FP32, tag=tag, bufs=nbufs,
                               name=f"proj{w_name}{mt}{n}")
                for k in range(N_HB):
                    nc.tensor.matmul(
                        ps,
                        lhsT=w_sb[w_name][:, k, mt * P:(mt + 1) * P],
                        rhs=xT[:, k, n * QM:(n + 1) * QM],
                        start=(k == 0),
                        stop=(k == N_HB - 1),
                    )
                nc.vector.tensor_scalar_add(
                    out=dst[:, mt, n * QM:(n + 1) * QM],
                    in0=ps,
                    scalar1=bias[:, mt:mt + 1],
                )

        # ---- projection granule queue: one matmul of some head-pair's
        # Q/K projection, fed into the attention loop's PE slack. Streams
        # of 8 granules share a PSUM accumulator; streams alternate between
        # the two single-buffer tags ("pjp"/"pjv") for double buffering.
        from collections import deque
        proj_q = deque()

        def queue_hp_proj(hp_t):
            for n in range(N_QM):
                for w_name in ("q", "k"):
                    for ki in range(N_HB):
                        proj_q.append((w_name, hp_t, n, ki))

        proj_state = {"tile": None, "stream": 0}

        def pop_proj():
            if not proj_q:
                return
            w_name, hp_t, n, ki = proj_q.popleft()
            if ki == 0:
                tag = "pjp" if (proj_state["stream"] % 2 == 0) else "pjv"
                proj_state["stream"] += 1
                proj_state["tile"] = ps_pj.tile(
                    [P, QM], FP32, tag=tag, bufs=1,
                    name=f"pj{w_name}{hp_t}{n}")
            ps = proj_state["tile"]
            nc.tensor.matmul(
                ps,
                lhsT=w_sb[w_name][:, ki, hp_t * P:(hp_t + 1) * P],
                rhs=xT[:, ki, n * QM:(n + 1) * QM],
                start=(ki == 0),
                stop=(ki == N_HB - 1),
            )
            if ki == N_HB - 1:
                dst, bias = (qT, bqT) if w_name == "q" else (kT, bkT)
                nc.vector.tensor_scalar_add(
                    out=dst[:, hp_t, n * QM:(n + 1) * QM],
                    in0=ps,
                    scalar1=bias[:, hp_t:hp_t + 1],
                )

        def st_transpose(st):
            xt = xt_tiles[st]
            for half in range(2):
                ps = ps_sc.tile([P, 4, P], FP32, tag="sc", name=f"xt{st}{half}")
                for q in range(4):
                    hb = half * 4 + q
                    nc.tensor.transpose(
                        ps[:, q, :], xt[:, hb * P:(hb + 1) * P], ident_f)
                nc.vector.tensor_copy(
                    out=xT[:, half * 4:half * 4 + 4, st * P:(st + 1) * P],
                    in_=ps,
                )

        def st_vproj(st):
            psv = ps_pj.tile([P, HG], FP32, tag="pjv", bufs=1, name=f"v{st}")
            for hb in range(N_HB):
                nc.tensor.matmul(
                    psv,
                    lhsT=xT[:, hb, st * P:(st + 1) * P],
                    rhs=w_sb["v"][:, hb, :],
                    start=(hb == 0),
                    stop=(hb == N_HB - 1),
                )
            nc.vector.scalar_tensor_tensor(
                out=vp[:, st, :, 0:HD],
                in0=psv.rearrange("p (h d) -> p h d", h=HEADS_PER_CORE),
                scalar=1.0,
                in1=bvb.rearrange("p (h d) -> p h d", h=HEADS_PER_CORE),
                op0=mybir.AluOpType.mult,
                op1=mybir.AluOpType.add,
            )

        def phase1_head():
            # Only x tiles 0-3 run here; 4-15 are granule-interleaved into
            # the first attention q-macro (phase1_merged_qm0). Critical path
            # to the first exp: x0-3 -> transposes -> proj(0,0), gated only
            # by the hp0 wq/wk column casts. Everything that waits on wv
            # (V', vp) is emitted after proj(0,0) so the PE's 4-deep wait
            # queue never wedges behind it.
            for st in range(4):
                st_transpose(st)
            cast_w("q")
            cast_w("k")
            proj_chunk(0, 0, pool=ps_ctx, tag="ctx")
            cast_w("v")
            for st in range(3):
                st_vproj(st)

        # ---- attention for one head pair -------------------------------
        PD = 80  # 65 padded to a multiple of XBAR_TILE_SRC_ROWS (16)

        def new_ctx_ps(hp, qm):
            return [ps_ctx.tile([HD + 1, QM], FP32, tag="ctx",
                                name=f"ctx{hp}{qm}{hh}")
                    for hh in range(2)]

        def attn_kc(hp, qm, kc, ctx_ps, nproj=1):
            for _ in range(nproj):
                pop_proj()
            sc = ps_sc.tile([P, 2, QM], FP32, tag="sc",
                            name=f"sc{hp}{qm}{kc}")
            for hh in range(2):
                lo = hh * HD
                nc.tensor.matmul(
                    sc[:, hh, :],
                    lhsT=kT[lo:lo + HD, hp, kc * P:(kc + 1) * P],
                    rhs=qT[lo:lo + HD, hp, qm * QM:(qm + 1) * QM],
                    start=True,
                    stop=True,
                    tile_position=(lo, 0),
                )
            et = expp.tile([P, 2, QM], FP16, tag="exp")
            nc.scalar.activation(
                out=et, in_=sc,
                func=mybir.ActivationFunctionType.Exp,
                scale=0.125,
            )
            for hh in range(2):
                nc.tensor.matmul(
                    ctx_ps[hh],
                    lhsT=vp[:, kc, 2 * hp + hh, :],
                    rhs=et[:, hh, :],
                    start=(kc == 0),
                    stop=(kc == N_KC - 1),
                )

        def epilogue_out(hp, qm, qs, srcs):
            ot = outp.tile([P, 2, HD], FP32, tag="out")
            for hh in range(2):
                rc = outp.tile([P, 1], FP32, tag="recip")
                nc.vector.reciprocal(out=rc, in_=srcs[hh][:, HD:HD + 1])
                nc.vector.tensor_scalar_mul(
                    ot[:, hh, :], srcs[hh][:, 0:HD], rc)
            row = qm * QM + qs * P
            col = 2 * hp * HD
            eng = nc.sync if (qs % 2) else nc.gpsimd
            eng.dma_start(
                out=out_d.ap()[row:row + P, col:col + 2 * HD],
                in_=ot,
            )

        def epilogue(hp, qm, ctx_ps, use_pe):
            if use_pe:
                # kernel tail: pipeline per q-slice so the first out DMA
                # fires right after the last ctx matmul instead of ~5us on.
                for qs in range(QM // P):
                    srcs = []
                    for hh in range(2):
                        csb = epil.tile([HD + 1, P], FP16,
                                        tag=f"ctxs{hh}", bufs=2)
                        nc.vector.tensor_copy(
                            out=csb, in_=ctx_ps[hh][:, qs * P:(qs + 1) * P])
                        tp = ps_pj.tile(
                            [P, HD + 2], FP16,
                            tag=("pjp" if hh == 0 else "pjv"), bufs=1,
                            name=f"tp{qs}{hh}")
                        nc.tensor.transpose(tp[:, 0:HD + 1], csb, ident_h)
                        srcs.append(tp)
                    epilogue_out(hp, qm, qs, srcs)
                return
            tsrcs = []
            for hh in range(2):
                csb = epil.tile([PD, QM], FP16, tag="ctxsb")
                nc.vector.tensor_copy(out=csb[0:HD + 1, :], in_=ctx_ps[hh])
                tpsb = epil.tile([P, QM // P, PD], FP16, tag="tpsb")
                for qs in range(QM // P):
                    nc.sync.dma_start_transpose(
                        out=tpsb[:, qs, :],
                        in_=csb[:, qs * P:(qs + 1) * P],
                    )
                tsrcs.append([tpsb[:, qs, :] for qs in range(QM // P)])
            for qs in range(QM // P):
                epilogue_out(hp, qm, qs, [tsrcs[0][qs], tsrcs[1][qs]])

        def phase1_merged_qm0():
            # First q-macro of head-pair 0, with the rest of the x pipeline
            # (tiles 4-15) granule-interleaved: the exp train starts ~60us
            # earlier while the PE grinds through transposes/V'/projections.
            # Schedule constraints: scores(kc) read the kT/qT chunk kc//4,
            # so proj_chunk(0, n) must precede kc = 4n; ctx(kc) reads vp[kc];
            # transposes are paced by the x DMA stream.
            # granule schedule (tr = transpose, v = V' projection); chosen
            # to satisfy: scores(kc) need proj(0, kc//4) emitted earlier,
            # ctx(kc) needs v(kc) emitted earlier, tr(st) is x-DMA-paced,
            # and the per-iteration PE load stays as even as possible.
            TR = {0: (4, 5), 1: (6,), 2: (7,), 4: (8,), 5: (9, 10),
                  6: (11,), 8: (12,), 9: (13,), 10: (14,), 11: (15,)}
            VP = {1: (3,), 2: (4,), 3: (5,), 4: (6,), 5: (7,), 6: (8, 9),
                  8: (10,), 9: (11,), 10: (12,), 11: (13,), 12: (14,),
                  13: (15,)}
            PJ = {3: 1, 7: 2, 11: 3}
            ctx_ps = new_ctx_ps(0, 0)
            for kc in range(N_KC):
                attn_kc(0, 0, kc, ctx_ps, nproj=0)
                for st in TR.get(kc, ()):
                    st_transpose(st)
                for st in VP.get(kc, ()):
                    st_vproj(st)
                if kc in PJ:
                    proj_chunk(0, PJ[kc])
            epilogue(0, 0, ctx_ps, use_pe=False)

        def attention(hp, qm_start=0):
            for qm in range(qm_start, N_QM):
                ctx_ps = new_ctx_ps(hp, qm)
                for kc in range(N_KC):
                    # catch-up rate 4/3 while hp1's projections must fit in
                    # hp0's remaining three q-macros (qm0 had no PE slack).
                    extra = 1 if (hp == 0 and kc % 3 == 0) else 0
                    attn_kc(hp, qm, kc, ctx_ps, nproj=1 + extra)
                epilogue(hp, qm, ctx_ps,
                         use_pe=(hp == N_MT - 1 and qm == N_QM - 1))

        phase1_head()
        queue_hp_proj(1)
        phase1_merged_qm0()
        attention(0, qm_start=1)
        for hp in range(1, N_MT):
            queue_hp_proj(hp + 1) if hp + 1 < N_MT else None
            attention(hp)


_NC_CACHE = None


def _get_nc():
    global _NC_CACHE
    if _NC_CACHE is None:
        _NC_CACHE = build()
    return _NC_CACHE


def make_in_maps(hidden_states, Wq, bq, Wk, bk, Wv, bv):
    hs = np.ascontiguousarray(np.asarray(hidden_states, dtype=np.float32))
    ws = {k: np.asarray(v, dtype=np.float32)
          for k, v in (("q", Wq), ("k", Wk), ("v", Wv))}
    bs = {k: np.asarray(v, dtype=np.float32)
          for k, v in (("q", bq), ("k", bk), ("v", bv))}
    in_maps = []
    for c in range(NCORES):
        b, g = c // 2, c % 2
        sl = slice(g * HG, (g + 1) * HG)
        in_maps.append({
            "x": np.ascontiguousarray(hs[b]),
            "wq": np.ascontiguousarray(ws["q"][:, sl]),
            "wk": np.ascontiguousarray(ws["k"][:, sl]),
            "wv": np.ascontiguousarray(ws["v"][:, sl]),
            "bq": np.ascontiguousarray(bs["q"][sl]),
            "bk": np.ascontiguousarray(bs["k"][sl]),
            "bv": np.ascontiguousarray(bs["v"][sl]),
        })
    return in_maps


def run(in_maps, trace=False):
    _ensure_profile_hook()
    nc = _get_nc()
    return run_bass_kernel_spmd(nc, in_maps, list(range(NCORES)), trace=trace)


def kernel(hidden_states, Wq, bq, Wk, bk, Wv, bv):
    in_maps = make_in_maps(hidden_states, Wq, bq, Wk, bk, Wv, bv)
    res = run(in_maps, trace=False)
    out = np.empty((B, S, H), dtype=np.float32)
    for c in range(NCORES):
        b, g = c // 2, c % 2
        out[b, :, g * HG:(g + 1) * HG] = res.results[c]["out"]
    return out


# revision 40
# speedup vs baseline: 1.0816x; 1.0816x over previous
"""BERT self-attention (B=4, S=2048, H=1024, 16 heads x 64) on 8 TRN2 NeuronCores.

Sharding: data-parallel over batch (4) x tensor-parallel over head-groups (2).
Core c handles batch c//2 and heads [8*(c%2), 8*(c%2)+8): it gets the full
hidden_states[b] plus the 512 W-columns/bias entries for its heads, and
produces out[b, :, 512*g : 512*(g+1)]. No cross-core communication.

Per-core kernel (bf16/fp16 matmuls, f32 accumulation in PSUM):
  phase1: all weight DMAs issued upfront (wv first) on the gpsimd queue,
    all x tiles on the sync queue. x is transposed by the PE in f32
    (2 cyc/row); the DVE PSUM->SBUF drain does the f32->bf16 cast for
    free, so the scalar engine (ACT) does no phase-1 work at all.
    Weight casts f32->bf16 run on the DVE, ordered wv -> st-chain -> wq/wk
    so they never delay the xT pipeline.
  attention (per head-pair, q-macro 512, k-chunk 128):
    scoresT[k, q] = KT_h[:, kc].T @ QT_h[:, qm]   (two heads row-packed, K=64)
    expT = exp(0.125 * scoresT)                   (ACT, fp16 out, N=1024/inst)
    ctxT[65, q] += V'_h[kc].T @ expT              (row 64 = softmax denominator)
    + one Q/K projection matmul of the NEXT head-pair in the PE slack.
    The loop is ACT-paced at ~1.34us/k-chunk (256 activations is the floor
    given the 8-bank PSUM budget: sc 2x2 + ctx 2x1 + pj 2x1).
  epilogue: ctxT -> DMA-transpose -> [q, 65]; reciprocal-multiply by the
  denominator; one batched [128, 128] DMA per q-slice covering both heads,
  alternating between the sync and gpsimd queues. (Final q-macro uses a PE
  transpose to keep the kernel tail short.)
"""

import sys
import types

sys.path.insert(0, "/opt/trn_rl_repo")

import numpy as np

import concourse.bass as bass
import concourse.tile as tile
from concourse import bacc, mybir
from concourse.bass_utils import run_bass_kernel_spmd
from concourse.masks import make_identity

B, S, H = 4, 2048, 1024
NH, HD = 16, 64
NCORES = 8
HEADS_PER_CORE = NH // 2      # 8 heads per core
HG = HEADS_PER_CORE * HD      # 512 = per-core head width
P = 128
QM = 512                      # q macro-tile
N_QM = S // QM                # 4
N_KC = S // P                 # 16 k chunks
N_ST = S // P                 # 16 s tiles
N_HB = H // P                 # 8 h chunks (contraction)
N_MT = HG // P                # 4 hd m-tiles

FP32 = mybir.dt.float32
BF16 = mybir.dt.bfloat16
FP16 = mybir.dt.float16


def _ensure_profile_hook():
    """The image's antenv lacks axon_hooks; shim it so trace=True works."""
    try:
        from antenv.axon_hooks import get_axon_ntff_profile_hook  # noqa: F401
        return
    except ImportError:
        pass
    try:
        from trn_agent_boot.trn_boot import _ntff_profile_via_ctypes
    except ImportError:
        return
    hook = _ntff_profile_via_ctypes("/opt/axon/libaxon_pjrt.so")
    mod = types.ModuleType("antenv.axon_hooks")
    mod.get_axon_ntff_profile_hook = lambda: hook
    mod.set_axon_ntff_profile_hook = lambda h: None
    sys.modules["antenv.axon_hooks"] = mod


def build():
    nc = bacc.Bacc("TRN2", target_bir_lowering=False, debug=False,
                   num_devices=NCORES)

    x_d = nc.declare_dram_parameter("x", [S, H], FP32, isOutput=False)
    wq_d = nc.declare_dram_parameter("wq", [H, HG], FP32, isOutput=False)
    wk_d = nc.declare_dram_parameter("wk", [H, HG], FP32, isOutput=False)
    wv_d = nc.declare_dram_parameter("wv", [H, HG], FP32, isOutput=False)
    bq_d = nc.declare_dram_parameter("bq", [HG], FP32, isOutput=False)
    bk_d = nc.declare_dram_parameter("bk", [HG], FP32, isOutput=False)
    bv_d = nc.declare_dram_parameter("bv", [HG], FP32, isOutput=False)
    out_d = nc.declare_dram_parameter("out", [S, HG], FP32, isOutput=True)

    with tile.TileContext(nc) as tc:
        _build_body(nc, tc, x_d, (wq_d, wk_d, wv_d), (bq_d, bk_d, bv_d), out_d)

    nc.finalize()
    return nc


def _build_body(nc, tc, x_d, w_d, b_d, out_d):
    wq_d, wk_d, wv_d = w_d
    bq_d, bk_d, bv_d = b_d

    import contextlib
    ctx = contextlib.ExitStack()
    with ctx:
        const = ctx.enter_context(tc.tile_pool(name="const", bufs=1))
        xf = ctx.enter_context(tc.tile_pool(name="xf", bufs=5))
        big = ctx.enter_context(tc.tile_pool(name="big", bufs=1))
        wstage = ctx.enter_context(tc.tile_pool(name="wstage", bufs=10))
        expp = ctx.enter_context(tc.tile_pool(name="expp", bufs=12))
        epil = ctx.enter_context(tc.tile_pool(name="epil", bufs=3))
        outp = ctx.enter_context(tc.tile_pool(name="outp", bufs=8))
        # created last: the exp tiles' SBUF address (expp) is performance-
        # critical (ACT activation speed is layout sensitive), so new pools
        # must not shift it.
        xbp = ctx.enter_context(tc.tile_pool(name="xbp", bufs=4))
        # PSUM budget (8 banks): ps_sc = 2 x 2-bank slots (scores double
        # buffer; also hosts the f32 transpose tiles in phase 1), ps_ctx =
        # 2 x 1-bank slots (ctx accumulators), ps_pj = 2 x 1-bank slots
        # (V'/QK projection accumulators).
        ps_sc = ctx.enter_context(
            tc.tile_pool(name="ps_sc", bufs=2, space="PSUM"))
        ps_ctx = ctx.enter_context(
            tc.tile_pool(name="ps_ctx", bufs=2, space="PSUM"))
        ps_pj = ctx.enter_context(
            tc.tile_pool(name="ps_pj", bufs=2, space="PSUM"))

        # ---- constants -------------------------------------------------
        ident_f = const.tile([P, P], FP32)
        make_identity(nc, ident_f)
        ident_h = const.tile([HD + 1, HD + 1], FP16)
        make_identity(nc, ident_h)

        # ---- x: all DMAs upfront on the sync queue, x0 leading ----------
        xt_tiles = []
        for st in range(N_ST):
            xt = xf.tile([P, H], FP32, tag="x", name=f"x{st}")
            nc.sync.dma_start(out=xt, in_=x_d.ap()[st * P:(st + 1) * P, :])
            xt_tiles.append(xt)

        bqT = const.tile([P, N_MT], FP32)
        nc.gpsimd.dma_start(out=bqT, in_=bq_d.ap().rearrange("(o p) -> p o", p=P))
        bkT = const.tile([P, N_MT], FP32)
        nc.gpsimd.dma_start(out=bkT, in_=bk_d.ap().rearrange("(o p) -> p o", p=P))
        bv_ap = bv_d.ap()
        bvb = const.tile([P, HG], FP32)
        nc.gpsimd.dma_start(
            out=bvb,
            in_=bass.AP(tensor=bv_ap.tensor, offset=bv_ap.offset,
                        ap=[[0, P]] + [list(a) for a in bv_ap.ap]),
        )

        # ---- weights: every DMA is issued upfront on the gpsimd queue.
        # wv comes first in row-chunks (V' consumes it within microseconds).
        # wq/wk are transferred in head-pair column slices so that only the
        # hp0 slice gates the first attention q-macro; the other slices
        # stream and cast (on the DVE) underneath the merged q-macro.
        w_sb = {}
        w_stg = {}
        for name in ("v", "q", "k"):
            w_sb[name] = big.tile([P, N_HB, HG], BF16, tag=f"w{name}",
                                  name=f"w{name}")

        # row-chunk DMAs (2KB lines -- column slices would be ~10x slower
        # per byte). wq/wk first: they gate the first attention q-macro.
        for name, wd in (("q", wq_d), ("k", wk_d), ("v", wv_d)):
            for hb in range(N_HB):
                stg = wstage.tile([P, HG], FP32, tag="wstg", bufs=10,
                                  name=f"w{name}{hb}")
                nc.gpsimd.dma_start(
                    out=stg, in_=wd.ap()[hb * P:(hb + 1) * P, :])
                w_stg[(name, hb)] = stg

        def cast_w(name):
            for hb in range(N_HB):
                nc.vector.tensor_copy(out=w_sb[name][:, hb, :],
                                      in_=w_stg.pop((name, hb)))

        xT = big.tile([P, N_HB, S], BF16, tag="xT")
        vp = big.tile([P, N_ST, HEADS_PER_CORE, HD + 1], BF16, tag="vp")
        # only the denominator ones-column needs initializing; columns 0:HD
        # are fully overwritten by the V' bias-add drains.
        nc.vector.memset(vp[:, :, :, HD:HD + 1], 1.0)

        qT = big.tile([P, N_MT, S], BF16, tag="qT")
        kT = big.tile([P, N_MT, S], BF16, tag="kT")

        def proj_chunk(mt, n, pool=None, tag=None, nbufs=None):
            pool = pool or ps_pj
            tag = tag or "pjp"
            if pool is ps_pj:
                nbufs = 1
            for w_name, dst, bias in (("q", qT, bqT), ("k", kT, bkT)):
                ps = pool.tile([P, QM], FP32, tag=tag, bufs=nbufs,
                               name=f"proj{w_name}{mt}{n}")
                for k in range(N_HB):
                    nc.tensor.matmul(
                        ps,
                        lhsT=w_sb[w_name][:, k, mt * P:(mt + 1) * P],
                        rhs=xT[:, k, n * QM:(n + 1) * QM],
                        start=(k == 0),
                        stop=(k == N_HB - 1),
                    )
                nc.vector.tensor_scalar_add(
                    out=dst[:, mt, n * QM:(n + 1) * QM],
                    in0=ps,
                    scalar1=bias[:, mt:mt + 1],
                )

        # ---- projection granule queue: one matmul of some head-pair's
        # Q/K projection, fed into the attention loop's PE slack. Streams
        # of 8 granules share a PSUM accumulator; streams alternate between
        # the two single-buffer tags ("pjp"/"pjv") for double buffering.
        from collections import deque
        proj_q = deque()

        def queue_hp_proj(hp_t):
            for n in range(N_QM):
                for w_name in ("q", "k"):
                    for ki in range(N_HB):
                        proj_q.append((w_name, hp_t, n, ki))

        proj_state = {"tile": None, "stream": 0}

        def pop_proj():
            if not proj_q:
                return
            w_name, hp_t, n, ki = proj_q.popleft()
            if ki == 0:
                tag = "pjp" if (proj_state["stream"] % 2 == 0) else "pjv"
                proj_state["stream"] += 1
                proj_state["tile"] = ps_pj.tile(
                    [P, QM], FP32, tag=tag, bufs=1,
                    name=f"pj{w_name}{hp_t}{n}")
            ps = proj_state["tile"]
            nc.tensor.matmul(
                ps,
                lhsT=w_sb[w_name][:, ki, hp_t * P:(hp_t + 1) * P],
                rhs=xT[:, ki, n * QM:(n + 1) * QM],
                start=(ki == 0),
                stop=(ki == N_HB - 1),
            )
            if ki == N_HB - 1:
                dst, bias = (qT, bqT) if w_name == "q" else (kT, bkT)
                nc.vector.tensor_scalar_add(
                    out=dst[:, hp_t, n * QM:(n + 1) * QM],
                    in0=ps,
                    scalar1=bias[:, hp_t:hp_t + 1],
                )

        def st_transpose(st):
            xt = xt_tiles[st]
            for half in range(2):
                ps = ps_sc.tile([P, 4, P], FP32, tag="sc", name=f"xt{st}{half}")
                for q in range(4):
                    hb = half * 4 + q
                    nc.tensor.transpose(
                        ps[:, q, :], xt[:, hb * P:(hb + 1) * P], ident_f)
                nc.vector.tensor_copy(
                    out=xT[:, half * 4:half * 4 + 4, st * P:(st + 1) * P],
                    in_=ps,
                )

        def st_vproj(st):
            psv = ps_pj.tile([P, HG], FP32, tag="pjv", bufs=1, name=f"v{st}")
            for hb in range(N_HB):
                nc.tensor.matmul(
                    psv,
                    lhsT=xT[:, hb, st * P:(st + 1) * P],
                    rhs=w_sb["v"][:, hb, :],
                    start=(hb == 0),
                    stop=(hb == N_HB - 1),
                )
            nc.vector.scalar_tensor_tensor(
                out=vp[:, st, :, 0:HD],
                in0=psv.rearrange("p (h d) -> p h d", h=HEADS_PER_CORE),
                scalar=1.0,
                in1=bvb.rearrange("p (h d) -> p h d", h=HEADS_PER_CORE),
                op0=mybir.AluOpType.mult,
                op1=mybir.AluOpType.add,
            )

        def phase1_head():
            # Only x tiles 0-3 run here; 4-15 are granule-interleaved into
            # the first attention q-macro (phase1_merged_qm0). Critical path
            # to the first exp: x0-3 -> transposes -> proj(0,0), gated only
            # by the hp0 wq/wk column casts. Everything that waits on wv
            # (V', vp) is emitted after proj(0,0) so the PE's 4-deep wait
            # queue never wedges behind it.
            for st in range(4):
                st_transpose(st)
            cast_w("q")
            cast_w("k")
            proj_chunk(0, 0, pool=ps_ctx, tag="ctx")
            cast_w("v")
            for st in range(3):
                st_vproj(st)

        # ---- attention for one head pair -------------------------------
        PD = 80  # 65 padded to a multiple of XBAR_TILE_SRC_ROWS (16)

        def new_ctx_ps(hp, qm):
            return [ps_ctx.tile([HD + 1, QM], FP32, tag="ctx",
                                name=f"ctx{hp}{qm}{hh}")
                    for hh in range(2)]

        def attn_kc(hp, qm, kc, ctx_ps, nproj=1):
            for _ in range(nproj):
                pop_proj()
            sc = ps_sc.tile([P, 2, QM], FP32, tag="sc",
                            name=f"sc{hp}{qm}{kc}")
            for hh in range(2):
                lo = hh * HD
                nc.tensor.matmul(
                    sc[:, hh, :],
                    lhsT=kT[lo:lo + HD, hp, kc * P:(kc + 1) * P],
                    rhs=qT[lo:lo + HD, hp, qm * QM:(qm + 1) * QM],
                    start=True,
                    stop=True,
                    tile_position=(lo, 0),
                )
            et = expp.tile([P, 2, QM], FP16, tag="exp")
            nc.scalar.activation(
                out=et, in_=sc,
                func=mybir.ActivationFunctionType.Exp,
                scale=0.125,
            )
            for hh in range(2):
                nc.tensor.matmul(
                    ctx_ps[hh],
                    lhsT=vp[:, kc, 2 * hp + hh, :],
                    rhs=et[:, hh, :],
                    start=(kc == 0),
                    stop=(kc == N_KC - 1),
                )

        def epilogue_out(hp, qm, qs, srcs):
            ot = outp.tile([P, 2, HD], FP32, tag="out")
            for hh in range(2):
                rc = outp.tile([P, 1], FP32, tag="recip")
                nc.vector.reciprocal(out=rc, in_=srcs[hh][:, HD:HD + 1])
                nc.vector.tensor_scalar_mul(
                    ot[:, hh, :], srcs[hh][:, 0:HD], rc)
            row = qm * QM + qs * P
            col = 2 * hp * HD
            eng = nc.sync if (qs % 2) else nc.gpsimd
            eng.dma_start(
                out=out_d.ap()[row:row + P, col:col + 2 * HD],
                in_=ot,
            )

        def epilogue(hp, qm, ctx_ps, use_pe):
            if use_pe:
                # kernel tail: pipeline per q-slice so the first out DMA
                # fires right after the last ctx matmul instead of ~5us on.
                for qs in range(QM // P):
                    srcs = []
                    for hh in range(2):
                        csb = epil.tile([HD + 1, P], FP16,
                                        tag=f"ctxs{hh}", bufs=2)
                        nc.vector.tensor_copy(
                            out=csb, in_=ctx_ps[hh][:, qs * P:(qs + 1) * P])
                        tp = ps_pj.tile(
                            [P, HD + 2], FP16,
                            tag=("pjp" if hh == 0 else "pjv"), bufs=1,
                            name=f"tp{qs}{hh}")
                        nc.tensor.transpose(tp[:, 0:HD + 1], csb, ident_h)
                        srcs.append(tp)
                    epilogue_out(hp, qm, qs, srcs)
                return
            tsrcs = []
            for hh in range(2):
                csb = epil.tile([PD, QM], FP16, tag="ctxsb")
                nc.vector.tensor_copy(out=csb[0:HD + 1, :], in_=ctx_ps[hh])
                tpsb = epil.tile([P, QM // P, PD], FP16, tag="tpsb")
                for qs in range(QM // P):
                    nc.sync.dma_start_transpose(
                        out=tpsb[:, qs, :],
                        in_=csb[:, qs * P:(qs + 1) * P],
                    )
                tsrcs.append([tpsb[:, qs, :] for qs in range(QM // P)])
            for qs in range(QM // P):
                epilogue_out(hp, qm, qs, [tsrcs[0][qs], tsrcs[1][qs]])

        def phase1_merged_qm0():
            # First q-macro of head-pair 0, with the rest of the x pipeline
            # (tiles 4-15) granule-interleaved: the exp train starts ~60us
            # earlier while the PE grinds through transposes/V'/projections.
            # Schedule constraints: scores(kc) read the kT/qT chunk kc//4,
            # so proj_chunk(0, n) must precede kc = 4n; ctx(kc) reads vp[kc];
            # transposes are paced by the x DMA stream.
            # granule schedule (tr = transpose, v = V' projection); chosen
            # to satisfy: scores(kc) need proj(0, kc//4) emitted earlier,
            # ctx(kc) needs v(kc) emitted earlier, tr(st) is x-DMA-paced,
            # and the per-iteration PE load stays as even as possible.
            TR = {0: (4, 5), 1: (6,), 2: (7,), 4: (8,), 5: (9, 10),
                  6: (11,), 8: (12,), 9: (13,), 10: (14,), 11: (15,)}
            VP = {1: (3,), 2: (4,), 3: (5,), 4: (6,), 5: (7,), 6: (8, 9),
                  8: (10,), 9: (11,), 10: (12,), 11: (13,), 12: (14,),
                  13: (15,)}
            PJ = {3: 1, 7: 2, 11: 3}
            ctx_ps = new_ctx_ps(0, 0)
            for kc in range(N_KC):
                attn_kc(0, 0, kc, ctx_ps, nproj=0)
                for st in TR.get(kc, ()):
                    st_transpose(st)
                for st in VP.get(kc, ()):
                    st_vproj(st)
                if kc in PJ:
                    proj_chunk(0, PJ[kc])
            epilogue(0, 0, ctx_ps, use_pe=False)

        def attention(hp, qm_start=0):
            for qm in range(qm_start, N_QM):
                ctx_ps = new_ctx_ps(hp, qm)
                for kc in range(N_KC):
                    # catch-up rate 4/3 while hp1's projections must fit in
                    # hp0's remaining three q-macros (qm0 had no PE slack).
                    extra = 1 if (hp == 0 and kc % 3 == 0) else 0
                    attn_kc(hp, qm, kc, ctx_ps, nproj=1 + extra)
                epilogue(hp, qm, ctx_ps,
                         use_pe=(hp == N_MT - 1 and qm == N_QM - 1))

        phase1_head()
        queue_hp_proj(1)
        phase1_merged_qm0()
        attention(0, qm_start=1)
        for hp in range(1, N_MT):
            queue_hp_proj(hp + 1) if hp + 1 < N_MT else None
            attention(hp)


_NC_CACHE = None


def _get_nc():
    global _NC_CACHE
    if _NC_CACHE is None:
        _NC_CACHE = build()
    return _NC_CACHE


def make_in_maps(hidden_states, Wq, bq, Wk, bk, Wv, bv):
    hs = np.ascontiguousarray(np.asarray(hidden_states, dtype=np.float32))
    ws = {k: np.asarray(v, dtype=np.float32)
          for k, v in (("q", Wq), ("k", Wk), ("v", Wv))}
    bs = {k: np.asarray(v, dtype=np.float32)
          for k, v in (("q", bq), ("k", bk), ("v", bv))}
    in_maps = []
    for c in range(NCORES):
        b, g = c // 2, c % 2
        sl = slice(g * HG, (g + 1) * HG)
        in_maps.append({
            "x": np.ascontiguousarray(hs[b]),
            "wq": np.ascontiguousarray(ws["q"][:, sl]),
            "wk": np.ascontiguousarray(ws["k"][:, sl]),
            "wv": np.ascontiguousarray(ws["v"][:, sl]),
            "bq": np.ascontiguousarray(bs["q"][sl]),
            "bk": np.ascontiguousarray(bs["k"][sl]),
            "bv": np.ascontiguousarray(bs["v"][sl]),
        })
    return in_maps


def run(in_maps, trace=False):
    _ensure_profile_hook()
    nc = _get_nc()
    return run_bass_kernel_spmd(nc, in_maps, list(range(NCORES)), trace=trace)


def kernel(hidden_states, Wq, bq, Wk, bk, Wv, bv):
    in_maps = make_in_maps(hidden_states, Wq, bq, Wk, bk, Wv, bv)
    res = run(in_maps, trace=False)
    out = np.empty((B, S, H), dtype=np.float32)
    for c in range(NCORES):
        b, g = c // 2, c % 2
        out[b, :, g * HG:(g + 1) * HG] = res.results[c]["out"]
    return out
